# revision 4
# baseline (speedup 1.0000x reference)
"""MoE gate (DeepSeek-style noaux_tc routing) Trainium2 kernel, v2.

kernel(**inputs) takes the FULL unsharded inputs
  hidden_states [4, 4096, 7168] f32, weight [256, 7168] f32,
  e_score_correction_bias [256] f32
and returns the FULL outputs (topk_idx [16384, 8] int32,
topk_weight [16384, 8] float32), matching the jax reference.

Sharding: data-parallel over the 16384-token axis across 8 NeuronCores
(2048 tokens each); gate weight + bias replicated.

v2 design (vs baseline): the gating GEMM keeps the gate weight chunks
STATIONARY in the PE array and streams transposed activations as the
MOVING operand with a 512-token free dim, so each fp32r matmul runs
1 cycle/row and the internal weight loads hide under the previous
matmul. Logits accumulate transposed ([expert, token]) in PSUM and are
re-transposed per 128-token tile for the on-chip routing (DVE top-8).

Two gemm modes:
  f32r3s : exact 3-term fp32r decomposition (xr@wr + xe@wr + xr@we),
           logits match fp32 to ~1e-7 (same numerics as baseline).
  fp8corr: main term xr@wr in fp32r; the two small correction terms
           (xe@wr + x@we) are computed in fp8e4m3 with DoubleRow
           (2 h-chunks contracted per instruction, 0.5 cyc/row).
           Power-of-2 scalings keep fp8 in range:
             xrs = f32r(2^13 x), xe8 = fp8(2^13 x - xrs),
             xr8 = fp8(x), wr8 = fp8(2^6 wr), we8 = fp8(2^19 we)
           main = 2^13 (xr@wr); corr = 2^19 (xe@wr + x@we)
           logits*2^13 = main + 2^-6 corr; sigmoid applies scale 2^-13.
           Routing flips vs exact: ~1 token of 16384 (rel ~2e-3).
"""
import sys
sys.path.insert(0, "/opt/trn_rl_repo")
import numpy as np
import concourse.bass as bass
import concourse.tile as tile
from concourse import bacc, mybir

F32 = mybir.dt.float32
F32R = mybir.dt.float32r
FP8 = mybir.dt.float8e4
U32 = mybir.dt.uint32
I32 = mybir.dt.int32
AF = mybir.ActivationFunctionType
ALU = mybir.AluOpType
AX = mybir.AxisListType
DR = mybir.MatmulPerfMode.DoubleRow

H = 7168
E = 256
NG = 8          # expert groups
GS = E // NG    # group size (32)
NCH = H // 128  # 56 h-chunks
NQ = 14         # weight load slices
QC = NCH // NQ  # chunks per slice (4)
TG = 512        # tokens per group (GEMM moving free dim)
NTT = TG // 128  # token tiles per group (4)
QH = 1792       # h per quarter (DMA granularity)
CPQ = 14        # chunks per quarter
LAG = 2
BIG = 1.0e30
CORR_MODE = "dr"


def _build(t_core: int, gemm: str = "fp8corr", n_devices: int = 8,
           repeat: int = 1):
    """in: x [t_core, H] f32; wTr [H, E] f32; bias_b [128, E];
    iota_b [128, E]; ident [128, 128].
    out: idx_out [t_core, 8] i32, w_out [t_core, 8] f32."""
    assert t_core % TG == 0
    ngroups = t_core // TG
    fp8c = gemm == "fp8corr"
    SX = 8192.0 if fp8c else 1.0      # x-side scale (2^13)
    SWR8, SWE8 = 2.0 ** 6, 2.0 ** 19  # fp8 weight scales

    nc = bacc.Bacc("TRN2", target_bir_lowering=False, debug=False,
                   num_devices=n_devices)

    x_d = nc.dram_tensor("x", [t_core, H], F32, kind="ExternalInput")
    w_d = nc.dram_tensor("wTr", [H, E], F32, kind="ExternalInput")
    bias_d = nc.dram_tensor("bias_b", [128, E], F32, kind="ExternalInput")
    iota_d = nc.dram_tensor("iota_b", [128, E], F32, kind="ExternalInput")
    ident_d = nc.dram_tensor("ident", [128, 128], F32, kind="ExternalInput")
    idx_d = nc.dram_tensor("idx_out", [t_core, 8], I32, kind="ExternalOutput")
    wout_d = nc.dram_tensor("w_out", [t_core, 8], F32, kind="ExternalOutput")

    wview = w_d[:].rearrange("(c p) e -> p c e", p=128)  # [128, 56, 256]

    with tile.TileContext(nc) as tc:
        with (
            tc.tile_pool(name="const", bufs=1) as constp,
            tc.tile_pool(name="wstage", bufs=1) as wsp,
            tc.tile_pool(name="xstage", bufs=8 if fp8c else 6) as xsp,
            tc.tile_pool(name="xops", bufs=3) as xop,
            tc.tile_pool(name="ltt", bufs=4) as ltp,
            tc.tile_pool(name="route", bufs=2) as rp,
            tc.tile_pool(name="small", bufs=2) as sp,
            tc.tile_pool(name="mainps", bufs=1, space="PSUM") as mainps,
            tc.tile_pool(name="corrps", bufs=1, space="PSUM") as corrps,
            tc.tile_pool(name="tbps", bufs=3, space="PSUM") as tbps,
            tc.tile_pool(name="lgps", bufs=1, space="PSUM") as lgps,
        ):

            # ---- resident constants ----
            ident = constp.tile([128, 128], F32)
            nc.sync.dma_start(ident[:], ident_d[:])
            # PE warm-up against the HAM clock gate during first DMAs
            warm = tbps.tile([128, 512], F32, name="warm", tag="tb")
            for ww in range(24):
                nc.tensor.matmul(warm[:, 0:128], ident[:], ident[:],
                                 is_transpose=True)
            bias_sb = constp.tile([128, E], F32)
            nc.gpsimd.dma_start(bias_sb[:], bias_d[:])
            iota_sb = constp.tile([128, E], F32)
            nc.gpsimd.dma_start(iota_sb[:], iota_d[:])

            # ---- weights: 14 slices of 4 chunks on the ACT HWDGE ring ----
            wr_t = [constp.tile([128, QC, E], F32R, name=f"wr_{q}",
                                tag=f"wr_{q}") for q in range(NQ)]
            if fp8c:
                wr8_t = [constp.tile([128, QC, E], FP8, name=f"wr8_{q}",
                                     tag=f"wr8_{q}") for q in range(NQ)]
                we8_t = [constp.tile([128, QC, E], FP8, name=f"we8_{q}",
                                     tag=f"we8_{q}") for q in range(NQ)]
            else:
                we_t = [constp.tile([128, QC, E], F32R, name=f"we_{q}",
                                    tag=f"we_{q}") for q in range(NQ)]

            def w_load(q):
                stage = wsp.tile([128, QC * E], F32, tag="wstage",
                                 name=f"wstage_{q}")
                sview = stage[:].rearrange("p (c e) -> p c e", e=E)
                nc.scalar.dma_start(sview, wview[:, q * QC:(q + 1) * QC, :])
                nc.vector.tensor_copy(wr_t[q][:], sview)
                if fp8c:
                    diff = wsp.tile([128, QC * E], F32, tag="wdiff",
                                    name=f"wdiff_{q}")
                    dview = diff[:].rearrange("p (c e) -> p c e", e=E)
                    nc.vector.tensor_tensor(dview, sview,
                                            wr_t[q][:].bitcast(F32),
                                            op=ALU.subtract)
                    nc.scalar.mul(we8_t[q][:], dview, SWE8)
                    nc.scalar.mul(wr8_t[q][:], wr_t[q][:].bitcast(F32), SWR8)
                else:
                    nc.vector.tensor_tensor(we_t[q][:], sview,
                                            wr_t[q][:].bitcast(F32),
                                            op=ALU.subtract)

            def wr_sl(c, eh):
                return wr_t[c // QC][:, c % QC, 128 * eh:128 * (eh + 1)]

            def we_sl(c, eh):
                return we_t[c // QC][:, c % QC, 128 * eh:128 * (eh + 1)]

            def wr8_sl(p, eh):  # chunk pair (2p, 2p+1)
                c = 2 * p
                return wr8_t[c // QC][:, c % QC:c % QC + 2,
                                      128 * eh:128 * (eh + 1)]

            def we8_sl(p, eh):
                c = 2 * p
                return we8_t[c // QC][:, c % QC:c % QC + 2,
                                      128 * eh:128 * (eh + 1)]

            def emit_all():
                xtiles = {}       # (g, q, i) -> xstage tile
                chunk_ops = {}    # c -> (xrs, xe) / (xrs,)
                pair8 = {}        # p -> (xe8, xr8)
                route_q = []      # pending routing: (ltT pair, g)

                def emit_x_dma(g):
                    # one DMA per h-quarter covering all 4 token tiles
                    # (fewer dma_starts -> less queue fixed overhead)
                    for xq in range(4):
                        t = xsp.tile([128, NTT, QH], F32, tag="xstage",
                                     name=f"x_{g}_{xq}", bufs=2)
                        nc.sync.dma_start(
                            t[:],
                            x_d[TG * g:TG * (g + 1),
                                QH * xq:QH * (xq + 1)].rearrange(
                                    "(i p) h -> p i h", p=128))
                        xtiles[(g, xq)] = t

                def emit_t_copies(g, c):
                    xq, cc = c // CPQ, c % CPQ
                    tb = tbps.tile([128, 512], F32, tag="tb",
                                   name=f"tb_{g}_{c}")
                    for i in range(NTT):
                        nc.tensor.matmul(
                            tb[:, 128 * i:128 * (i + 1)],
                            xtiles[(g, xq)][:, i, 128 * cc:128 * (cc + 1)],
                            ident[:], is_transpose=True,
                            start=(i == 0), stop=(i == NTT - 1),
                            skip_group_check=True)
                    xrs = xop.tile([128, 512], F32R, tag="xrs",
                                   name=f"xrs_{g}_{c}")
                    nc.scalar.mul(xrs[:], tb[:], SX)
                    if fp8c:
                        p, slot = c // 2, c % 2
                        if slot == 0:
                            xe8 = xop.tile([128, 2, 512], FP8, tag="xe8",
                                           name=f"xe8_{g}_{p}", bufs=3)
                            xr8 = xop.tile([128, 2, 512], FP8, tag="xr8",
                                           name=f"xr8_{g}_{p}", bufs=3)
                            pair8[p] = (xe8, xr8)
                        xe8, xr8 = pair8[p]
                        nc.vector.scalar_tensor_tensor(
                            xe8[:, slot, :], tb[:], SX, xrs[:].bitcast(F32),
                            op0=ALU.mult, op1=ALU.subtract)
                        # xr8 = fp8(xr) from SBUF (keeps ACT off PSUM and
                        # lets tb recycle after two readers)
                        nc.scalar.mul(xr8[:, slot, :], xrs[:].bitcast(F32),
                                      1.0 / SX)
                        chunk_ops[c] = (xrs,)
                    else:
                        xe = xop.tile([128, 512], F32R, tag="xe",
                                      name=f"xe_{g}_{c}")
                        nc.vector.scalar_tensor_tensor(
                            xe[:], tb[:], SX, xrs[:].bitcast(F32),
                            op0=ALU.mult, op1=ALU.subtract)
                        chunk_ops[c] = (xrs, xe)

                def emit_main(g, cg, main):
                    ops = chunk_ops.pop(cg)
                    xrs = ops[0]
                    for eh in range(2):
                        if fp8c:
                            nc.tensor.matmul(
                                main[eh][:], wr_sl(cg, eh), xrs[:],
                                start=(cg == 0), stop=(cg == NCH - 1))
                        else:
                            xe = ops[1]
                            nc.tensor.matmul(
                                main[eh][:], wr_sl(cg, eh), xrs[:],
                                start=(cg == 0), stop=False)
                            nc.tensor.matmul(
                                main[eh][:], wr_sl(cg, eh), xe[:],
                                start=False, stop=False)
                            nc.tensor.matmul(
                                main[eh][:], we_sl(cg, eh), xrs[:],
                                start=False, stop=(cg == NCH - 1))

                def emit_corr(p, corr):
                    xe8, xr8 = pair8.pop(p)
                    if CORR_MODE == "none":
                        return
                    for eh in range(2):
                        if CORR_MODE == "dr":
                            nc.tensor.matmul(
                                corr[eh][:], wr8_sl(p, eh), xe8[:, 0:2, :],
                                perf_mode=DR, start=(p == 0), stop=False)
                            nc.tensor.matmul(
                                corr[eh][:], we8_sl(p, eh), xr8[:, 0:2, :],
                                perf_mode=DR, start=False,
                                stop=(p == NCH // 2 - 1))
                        else:  # plain fp8, 2 MMs per pair slot
                            for sl in range(2):
                                nc.tensor.matmul(
                                    corr[eh][:], wr8_sl(p, eh)[:, sl, :],
                                    xe8[:, sl, :],
                                    start=(p == 0 and sl == 0), stop=False)
                                nc.tensor.matmul(
                                    corr[eh][:], we8_sl(p, eh)[:, sl, :],
                                    xr8[:, sl, :], start=False,
                                    stop=(p == NCH // 2 - 1 and sl == 1))

                def emit_combine(g, main, corr):
                    ltT = []
                    for eh in range(2):
                        t = ltp.tile([128, 512], F32, tag=f"ltT{eh}",
                                     name=f"ltT_{g}_{eh}", bufs=2)
                        if fp8c:
                            # (two PSUM operands can't feed one instruction)
                            cs = ltp.tile([128, 512], F32, tag="corrsb",
                                          name=f"cs_{g}_{eh}", bufs=2)
                            nc.scalar.mul(cs[:], corr[eh][:], 1.0 / 64.0)
                            nc.vector.tensor_tensor(t[:], cs[:],
                                                    main[eh][:], op=ALU.add)
                        else:
                            nc.vector.tensor_copy(t[:], main[eh][:])
                        ltT.append(t)
                    route_q.append((g, ltT))

                rstate = {}
                gout = {}

                def emit_routing_piece(g, i, ltT, piece):
                    """Routing for token tile i of group g, split into 5
                    pieces emitted at consecutive chunk steps so the serial
                    dependency chain doesn't block the in-order engine
                    queues (chunk work interleaves between pieces)."""
                    st = rstate.setdefault((g, i), {})
                    if piece == 0:
                        lg = lgps.tile([128, E], F32, tag="lg",
                                       name=f"lg_{g}_{i}")
                        for eh in range(2):
                            nc.tensor.matmul(
                                lg[:, 128 * eh:128 * (eh + 1)],
                                ltT[eh][:, 128 * i:128 * (i + 1)], ident[:],
                                is_transpose=True, start=(eh == 0),
                                stop=(eh == 1), skip_group_check=True)
                        scores = rp.tile([128, E], F32, tag="scores",
                                         name=f"sc_{g}_{i}")
                        nc.scalar.activation(scores[:], lg[:], AF.Sigmoid,
                                             scale=1.0 / SX)
                        st["scores"] = scores
                    elif piece == 1:
                        scores = st["scores"]
                        sfc = rp.tile([128, E], F32, tag="sfc",
                                      name=f"sfc_{g}_{i}")
                        nc.vector.tensor_tensor(sfc[:], scores[:],
                                                bias_sb[:], op=ALU.add)
                        g8 = sp.tile([128, 64], F32, tag="g8",
                                     name=f"g8_{g}_{i}")
                        for gg in range(NG):
                            nc.vector.max(g8[:, 8 * gg:8 * gg + 8],
                                          sfc[:, GS * gg:GS * (gg + 1)])
                        gsc = sp.tile([128, NG], F32, tag="gsc",
                                      name=f"gsc_{g}_{i}")
                        nc.vector.tensor_reduce(
                            gsc[:],
                            g8[:].rearrange("p (g i) -> p g i",
                                            i=8)[:, :, 0:2],
                            axis=AX.X, op=ALU.add)
                        gt8 = sp.tile([128, 8], F32, tag="gt8",
                                      name=f"gt8_{g}_{i}")
                        nc.vector.max(gt8[:], gsc[:])
                        pen = sp.tile([128, NG], F32, tag="pen",
                                      name=f"pen_{g}_{i}")
                        nc.vector.tensor_scalar(pen[:], gsc[:], gt8[:, 3:4],
                                                -BIG, op0=ALU.is_lt,
                                                op1=ALU.mult)
                        masked = rp.tile([128, E], F32, tag="masked",
                                         name=f"mk_{g}_{i}")
                        for gg in range(NG):
                            nc.gpsimd.tensor_scalar_add(
                                masked[:, GS * gg:GS * (gg + 1)],
                                sfc[:, GS * gg:GS * (gg + 1)],
                                pen[:, gg:gg + 1])
                        st["sfc"], st["masked"] = sfc, masked
                    elif piece == 2:
                        masked = st["masked"]
                        m8 = sp.tile([128, 8], F32, tag="m8",
                                     name=f"m8_{g}_{i}")
                        nc.vector.max(m8[:], masked[:])
                        if g not in gout:
                            gout[g] = (
                                sp.tile([128, NTT, 8], U32, tag="oidx",
                                        name=f"oidx_{g}"),
                                sp.tile([128, NTT, 8], F32, tag="owout",
                                        name=f"owout_{g}"))
                        i8 = gout[g][0][:, i, :]
                        nc.vector.max_index(i8, m8[:], masked[:])
                        i8f = sp.tile([128, 8], F32, tag="i8f",
                                      name=f"i8f_{g}_{i}")
                        nc.vector.tensor_copy(i8f[:], i8)
                        st["m8"], st["i8"], st["i8f"] = m8, i8, i8f
                    elif piece == 3:
                        i8f = st["i8f"]
                        junk = rp.tile([128, E], F32, tag="junk",
                                       name=f"junk_{g}_{i}")
                        biasg = sp.tile([128, 8], F32, tag="biasg",
                                        name=f"biasg_{g}_{i}")
                        for k in range(8):
                            nc.vector.scalar_tensor_tensor(
                                junk[:], iota_sb[:], i8f[:, k:k + 1],
                                bias_sb[:], op0=ALU.is_equal, op1=ALU.mult,
                                accum_out=biasg[:, k:k + 1])
                        st["biasg"] = biasg
                    else:
                        m8, i8, biasg = st["m8"], st["i8"], st["biasg"]
                        wraw = sp.tile([128, 8], F32, tag="wraw",
                                       name=f"wraw_{g}_{i}")
                        nc.vector.tensor_tensor(wraw[:], m8[:], biasg[:],
                                                op=ALU.subtract)
                        ssum = sp.tile([128, 1], F32, tag="ssum",
                                       name=f"ssum_{g}_{i}")
                        nc.vector.tensor_reduce(ssum[:], wraw[:], axis=AX.X,
                                                op=ALU.add)
                        inv = sp.tile([128, 1], F32, tag="inv",
                                      name=f"inv_{g}_{i}")
                        nc.vector.reciprocal(inv[:], ssum[:])
                        nc.vector.tensor_scalar(gout[g][1][:, i, :],
                                                wraw[:], inv[:],
                                                2.5, op0=ALU.mult,
                                                op1=ALU.mult)
                        if i == NTT - 1:  # one batched DMA per output
                            oidx, owout = gout.pop(g)
                            nc.sync.dma_start(
                                idx_d[TG * g:TG * (g + 1), :].rearrange(
                                    "(i p) k -> p i k", p=128),
                                oidx[:].bitcast(I32))
                            nc.sync.dma_start(
                                wout_d[TG * g:TG * (g + 1), :].rearrange(
                                    "(i p) k -> p i k", p=128),
                                owout[:])
                        rstate.pop((g, i))

                # ttile i pieces at steps 6+12i .. 10+12i
                ROUTE_AT = {6 + 12 * i + p: (i, p)
                            for i in range(NTT) for p in range(5)}
                for q in range(5):
                    w_load(q)
                for g in range(ngroups):
                    emit_x_dma(g)
                    main = [mainps.tile([128, 512], F32, tag=f"main{eh}",
                                        name=f"main_{g}_{eh}")
                            for eh in range(2)]
                    corr = [corrps.tile([128, 512], F32, tag=f"corr{eh}",
                                        name=f"corr_{g}_{eh}")
                            for eh in range(2)] if fp8c else None
                    for s in range(NCH + LAG + 1):
                        if s < NCH:
                            if g == 0 and s % 2 == 0 and 5 + s // 2 < NQ:
                                w_load(5 + s // 2)
                            emit_t_copies(g, s)
                        cg = s - LAG
                        if 0 <= cg < NCH:
                            emit_main(g, cg, main)
                        if fp8c and cg >= 2 and cg % 2 == 0:
                            emit_corr(cg // 2 - 1, corr)
                        if route_q and s in ROUTE_AT:
                            gq, ltT = route_q[0]
                            i, p = ROUTE_AT[s]
                            emit_routing_piece(gq, i, ltT, p)
                            if s == max(ROUTE_AT):
                                route_q.pop(0)
                    emit_combine(g, main, corr)
                # drain last group's routing
                gq, ltT = route_q.pop(0)
                for i in range(NTT):
                    for p in range(5):
                        emit_routing_piece(gq, i, ltT, p)

            if repeat == 1:
                emit_all()
            else:
                with tc.For_i(0, repeat, 1):
                    emit_all()

    nc.compile()
    return nc


_NC_CACHE = {}
_T_FULL = 16384
_N_CORES = 8
_GEMM = "f32r3s"


def kernel(hidden_states, weight, e_score_correction_bias):
    from concourse.bass_utils import run_bass_kernel_spmd

    x = np.ascontiguousarray(
        np.asarray(hidden_states, dtype=np.float32).reshape(_T_FULL, H))
    w = np.asarray(weight, dtype=np.float32)
    bias = np.asarray(e_score_correction_bias, dtype=np.float32)
    t_core = _T_FULL // _N_CORES

    if _GEMM not in _NC_CACHE:
        _NC_CACHE[_GEMM] = _build(t_core, gemm=_GEMM, n_devices=_N_CORES)
    nc = _NC_CACHE[_GEMM]

    base = {
        "wTr": np.ascontiguousarray(w.T),
        "bias_b": np.ascontiguousarray(
            np.broadcast_to(bias[None, :], (128, E))),
        "iota_b": np.ascontiguousarray(
            np.broadcast_to(np.arange(E, dtype=np.float32)[None, :],
                            (128, E))),
        "ident": np.eye(128, dtype=np.float32),
    }
    maps = []
    for c in range(_N_CORES):
        m = dict(base)
        m["x"] = np.ascontiguousarray(x[c * t_core:(c + 1) * t_core])
        maps.append(m)

    br = run_bass_kernel_spmd(nc, maps, list(range(_N_CORES)))
    idx = np.concatenate(
        [br.results[c]["idx_out"] for c in range(_N_CORES)],
        axis=0).astype(np.int32)
    wout = np.concatenate(
        [br.results[c]["w_out"] for c in range(_N_CORES)],
        axis=0).astype(np.float32)
    return idx, wout


# revision 6
# speedup vs baseline: 1.3639x; 1.3639x over previous
"""MoE gate (DeepSeek-style noaux_tc routing) Trainium2 kernel, v2.

kernel(**inputs) takes the FULL unsharded inputs
  hidden_states [4, 4096, 7168] f32, weight [256, 7168] f32,
  e_score_correction_bias [256] f32
and returns the FULL outputs (topk_idx [16384, 8] int32,
topk_weight [16384, 8] float32), matching the jax reference.

Sharding: data-parallel over the 16384-token axis across 8 NeuronCores
(2048 tokens each); gate weight + bias replicated.

v2 design (vs baseline): the gating GEMM keeps the gate weight chunks
STATIONARY in the PE array and streams transposed activations as the
MOVING operand with a 512-token free dim, so each fp32r matmul runs
1 cycle/row and the internal weight loads hide under the previous
matmul. Logits accumulate transposed ([expert, token]) in PSUM and are
re-transposed per 128-token tile for the on-chip routing (DVE top-8).

Two gemm modes:
  f32r3s : exact 3-term fp32r decomposition (xr@wr + xe@wr + xr@we),
           logits match fp32 to ~1e-7 (same numerics as baseline).
  fp8corr: main term xr@wr in fp32r; the two small correction terms
           (xe@wr + x@we) are computed in fp8e4m3 with DoubleRow
           (2 h-chunks contracted per instruction, 0.5 cyc/row).
           Power-of-2 scalings keep fp8 in range:
             xrs = f32r(2^13 x), xe8 = fp8(2^13 x - xrs),
             xr8 = fp8(x), wr8 = fp8(2^6 wr), we8 = fp8(2^19 we)
           main = 2^13 (xr@wr); corr = 2^19 (xe@wr + x@we)
           logits*2^13 = main + 2^-6 corr; sigmoid applies scale 2^-13.
           Routing flips vs exact: ~1 token of 16384 (rel ~2e-3).
"""
import sys
sys.path.insert(0, "/opt/trn_rl_repo")
import numpy as np
import concourse.bass as bass
import concourse.tile as tile
from concourse import bacc, mybir

F32 = mybir.dt.float32
F32R = mybir.dt.float32r
FP8 = mybir.dt.float8e4
U32 = mybir.dt.uint32
I32 = mybir.dt.int32
AF = mybir.ActivationFunctionType
ALU = mybir.AluOpType
AX = mybir.AxisListType
DR = mybir.MatmulPerfMode.DoubleRow

H = 7168
E = 256
NG = 8          # expert groups
GS = E // NG    # group size (32)
NCH = H // 128  # 56 h-chunks
NQ = 14         # weight load slices
QC = NCH // NQ  # chunks per slice (4)
TG = 512        # tokens per group (GEMM moving free dim)
NTT = TG // 128  # token tiles per group (4)
QH = 1792       # h per quarter (DMA granularity)
CPQ = 14        # chunks per quarter
LAG = 2
BIG = 1.0e30
CORR_MODE = "dr"


def _build(t_core: int, gemm: str = "fp8corr", n_devices: int = 8,
           repeat: int = 1):
    """in: x [t_core, H] f32; wTr [H, E] f32; bias_b [128, E];
    iota_b [128, E]; ident [128, 128].
    out: idx_out [t_core, 8] i32, w_out [t_core, 8] f32."""
    assert t_core % TG == 0
    ngroups = t_core // TG
    fp8c = gemm == "fp8corr"
    SX = 8192.0 if fp8c else 1.0      # x-side scale (2^13)
    SWR8, SWE8 = 2.0 ** 6, 2.0 ** 19  # fp8 weight scales

    nc = bacc.Bacc("TRN2", target_bir_lowering=False, debug=False,
                   num_devices=n_devices)

    x_d = nc.dram_tensor("x", [t_core, H], F32, kind="ExternalInput")
    w_d = nc.dram_tensor("wTr", [H, E], F32, kind="ExternalInput")
    bias_d = nc.dram_tensor("bias_b", [128, E], F32, kind="ExternalInput")
    iota_d = nc.dram_tensor("iota_b", [128, E], F32, kind="ExternalInput")
    ident_d = nc.dram_tensor("ident", [128, 128], F32, kind="ExternalInput")
    idx_d = nc.dram_tensor("idx_out", [t_core, 8], I32, kind="ExternalOutput")
    wout_d = nc.dram_tensor("w_out", [t_core, 8], F32, kind="ExternalOutput")

    wview = w_d[:].rearrange("(c p) e -> p c e", p=128)  # [128, 56, 256]

    with tile.TileContext(nc) as tc:
        with (
            tc.tile_pool(name="const", bufs=1) as constp,
            tc.tile_pool(name="wstage", bufs=2) as wsp,
            tc.tile_pool(name="xstage", bufs=8 if fp8c else 6) as xsp,
            tc.tile_pool(name="xops", bufs=4) as xop,
            tc.tile_pool(name="ltt", bufs=4) as ltp,
            tc.tile_pool(name="route", bufs=2) as rp,
            tc.tile_pool(name="small", bufs=2) as sp,
            tc.tile_pool(name="mainps", bufs=1, space="PSUM") as mainps,
            tc.tile_pool(name="corrps", bufs=1, space="PSUM") as corrps,
            tc.tile_pool(name="tbps", bufs=3, space="PSUM") as tbps,
            tc.tile_pool(name="lgps", bufs=1, space="PSUM") as lgps,
        ):

            # ---- resident constants ----
            ident = constp.tile([128, 128], F32)
            nc.sync.dma_start(ident[:], ident_d[:])
            # PE warm-up against the HAM clock gate during first DMAs
            warm = tbps.tile([128, 512], F32, name="warm", tag="tb")
            for ww in range(24):
                nc.tensor.matmul(warm[:, 0:128], ident[:], ident[:],
                                 is_transpose=True)
            bias_sb = constp.tile([128, E], F32)
            nc.gpsimd.dma_start(bias_sb[:], bias_d[:])
            iota_sb = constp.tile([128, E], F32)
            nc.gpsimd.dma_start(iota_sb[:], iota_d[:])

            # ---- weights: 14 slices of 4 chunks on the ACT HWDGE ring ----
            wr_t = [constp.tile([128, QC, E], F32R, name=f"wr_{q}",
                                tag=f"wr_{q}") for q in range(NQ)]
            if fp8c:
                wr8_t = [constp.tile([128, QC, E], FP8, name=f"wr8_{q}",
                                     tag=f"wr8_{q}") for q in range(NQ)]
                we8_t = [constp.tile([128, QC, E], FP8, name=f"we8_{q}",
                                     tag=f"we8_{q}") for q in range(NQ)]
            else:
                we_t = [constp.tile([128, QC, E], F32R, name=f"we_{q}",
                                    tag=f"we_{q}") for q in range(NQ)]

            def w_load(q):
                stage = wsp.tile([128, QC * E], F32, tag="wstage",
                                 name=f"wstage_{q}")
                sview = stage[:].rearrange("p (c e) -> p c e", e=E)
                nc.scalar.dma_start(sview, wview[:, q * QC:(q + 1) * QC, :])
                nc.vector.tensor_copy(wr_t[q][:], sview)
                if fp8c:
                    diff = wsp.tile([128, QC * E], F32, tag="wdiff",
                                    name=f"wdiff_{q}")
                    dview = diff[:].rearrange("p (c e) -> p c e", e=E)
                    nc.vector.tensor_tensor(dview, sview,
                                            wr_t[q][:].bitcast(F32),
                                            op=ALU.subtract)
                    nc.scalar.mul(we8_t[q][:], dview, SWE8)
                    nc.scalar.mul(wr8_t[q][:], wr_t[q][:].bitcast(F32), SWR8)
                else:
                    nc.vector.tensor_tensor(we_t[q][:], sview,
                                            wr_t[q][:].bitcast(F32),
                                            op=ALU.subtract)

            def wr_sl(c, eh):
                return wr_t[c // QC][:, c % QC, 128 * eh:128 * (eh + 1)]

            def we_sl(c, eh):
                return we_t[c // QC][:, c % QC, 128 * eh:128 * (eh + 1)]

            def wr8_sl(p, eh):  # chunk pair (2p, 2p+1)
                c = 2 * p
                return wr8_t[c // QC][:, c % QC:c % QC + 2,
                                      128 * eh:128 * (eh + 1)]

            def we8_sl(p, eh):
                c = 2 * p
                return we8_t[c // QC][:, c % QC:c % QC + 2,
                                      128 * eh:128 * (eh + 1)]

            def emit_all():
                xtiles = {}       # (g, q, i) -> xstage tile
                chunk_ops = {}    # c -> (xrs, xe) / (xrs,)
                pair8 = {}        # p -> (xe8, xr8)
                route_q = []      # pending routing: (ltT pair, g)

                def emit_x_dma(g):
                    # one DMA per h-quarter covering all 4 token tiles
                    # (fewer dma_starts -> less queue fixed overhead)
                    for xq in range(4):
                        t = xsp.tile([128, NTT, QH], F32, tag="xstage",
                                     name=f"x_{g}_{xq}", bufs=2)
                        nc.sync.dma_start(
                            t[:],
                            x_d[TG * g:TG * (g + 1),
                                QH * xq:QH * (xq + 1)].rearrange(
                                    "(i p) h -> p i h", p=128))
                        xtiles[(g, xq)] = t

                def emit_t_copies(g, c):
                    xq, cc = c // CPQ, c % CPQ
                    tb = tbps.tile([128, 512], F32, tag="tb",
                                   name=f"tb_{g}_{c}")
                    for i in range(NTT):
                        nc.tensor.matmul(
                            tb[:, 128 * i:128 * (i + 1)],
                            xtiles[(g, xq)][:, i, 128 * cc:128 * (cc + 1)],
                            ident[:], is_transpose=True,
                            start=(i == 0), stop=(i == NTT - 1),
                            skip_group_check=True)
                    xrs = xop.tile([128, 512], F32R, tag="xrs",
                                   name=f"xrs_{g}_{c}")
                    nc.scalar.mul(xrs[:], tb[:], SX)
                    if fp8c:
                        p, slot = c // 2, c % 2
                        if slot == 0:
                            xe8 = xop.tile([128, 2, 512], FP8, tag="xe8",
                                           name=f"xe8_{g}_{p}", bufs=3)
                            xr8 = xop.tile([128, 2, 512], FP8, tag="xr8",
                                           name=f"xr8_{g}_{p}", bufs=3)
                            pair8[p] = (xe8, xr8)
                        xe8, xr8 = pair8[p]
                        nc.vector.scalar_tensor_tensor(
                            xe8[:, slot, :], tb[:], SX, xrs[:].bitcast(F32),
                            op0=ALU.mult, op1=ALU.subtract)
                        # xr8 = fp8(xr) from SBUF (keeps ACT off PSUM and
                        # lets tb recycle after two readers)
                        nc.scalar.mul(xr8[:, slot, :], xrs[:].bitcast(F32),
                                      1.0 / SX)
                        chunk_ops[c] = (xrs,)
                    else:
                        xe = xop.tile([128, 512], F32R, tag="xe",
                                      name=f"xe_{g}_{c}")
                        nc.vector.scalar_tensor_tensor(
                            xe[:], tb[:], SX, xrs[:].bitcast(F32),
                            op0=ALU.mult, op1=ALU.subtract)
                        chunk_ops[c] = (xrs, xe)

                def emit_main(g, cg, main):
                    ops = chunk_ops.pop(cg)
                    xrs = ops[0]
                    for eh in range(2):
                        if fp8c:
                            nc.tensor.matmul(
                                main[eh][:], wr_sl(cg, eh), xrs[:],
                                start=(cg == 0), stop=(cg == NCH - 1))
                        else:
                            xe = ops[1]
                            nc.tensor.matmul(
                                main[eh][:], wr_sl(cg, eh), xrs[:],
                                start=(cg == 0), stop=False)
                            nc.tensor.matmul(
                                main[eh][:], wr_sl(cg, eh), xe[:],
                                start=False, stop=False)
                            nc.tensor.matmul(
                                main[eh][:], we_sl(cg, eh), xrs[:],
                                start=False, stop=(cg == NCH - 1))

                def emit_corr(p, corr):
                    xe8, xr8 = pair8.pop(p)
                    if CORR_MODE == "none":
                        return
                    for eh in range(2):
                        if CORR_MODE == "dr":
                            nc.tensor.matmul(
                                corr[eh][:], wr8_sl(p, eh), xe8[:, 0:2, :],
                                perf_mode=DR, start=(p == 0), stop=False)
                            nc.tensor.matmul(
                                corr[eh][:], we8_sl(p, eh), xr8[:, 0:2, :],
                                perf_mode=DR, start=False,
                                stop=(p == NCH // 2 - 1))
                        else:  # plain fp8, 2 MMs per pair slot
                            for sl in range(2):
                                nc.tensor.matmul(
                                    corr[eh][:], wr8_sl(p, eh)[:, sl, :],
                                    xe8[:, sl, :],
                                    start=(p == 0 and sl == 0), stop=False)
                                nc.tensor.matmul(
                                    corr[eh][:], we8_sl(p, eh)[:, sl, :],
                                    xr8[:, sl, :], start=False,
                                    stop=(p == NCH // 2 - 1 and sl == 1))

                def emit_combine(g, main, corr):
                    ltT = []
                    for eh in range(2):
                        t = ltp.tile([128, 512], F32, tag=f"ltT{eh}",
                                     name=f"ltT_{g}_{eh}", bufs=2)
                        if fp8c:
                            # (two PSUM operands can't feed one instruction)
                            cs = ltp.tile([128, 512], F32, tag="corrsb",
                                          name=f"cs_{g}_{eh}", bufs=2)
                            nc.scalar.mul(cs[:], corr[eh][:], 1.0 / 64.0)
                            nc.vector.tensor_tensor(t[:], cs[:],
                                                    main[eh][:], op=ALU.add)
                        else:
                            nc.vector.tensor_copy(t[:], main[eh][:])
                        ltT.append(t)
                    route_q.append((g, ltT))

                rstate = {}
                gout = {}

                def emit_routing_piece(g, i, ltT, piece):
                    """Routing for token tile i of group g, split into 5
                    pieces emitted at consecutive chunk steps so the serial
                    dependency chain doesn't block the in-order engine
                    queues (chunk work interleaves between pieces)."""
                    st = rstate.setdefault((g, i), {})
                    if piece == 0:
                        lg = lgps.tile([128, E], F32, tag="lg",
                                       name=f"lg_{g}_{i}")
                        for eh in range(2):
                            nc.tensor.matmul(
                                lg[:, 128 * eh:128 * (eh + 1)],
                                ltT[eh][:, 128 * i:128 * (i + 1)], ident[:],
                                is_transpose=True, start=(eh == 0),
                                stop=(eh == 1), skip_group_check=True)
                        scores = rp.tile([128, E], F32, tag="scores",
                                         name=f"sc_{g}_{i}")
                        nc.scalar.activation(scores[:], lg[:], AF.Sigmoid,
                                             scale=1.0 / SX)
                        st["scores"] = scores
                    elif piece == 1:
                        scores = st["scores"]
                        sfc = rp.tile([128, E], F32, tag="sfc",
                                      name=f"sfc_{g}_{i}")
                        nc.vector.tensor_tensor(sfc[:], scores[:],
                                                bias_sb[:], op=ALU.add)
                        g8 = sp.tile([128, 64], F32, tag="g8",
                                     name=f"g8_{g}_{i}")
                        for gg in range(NG):
                            nc.vector.max(g8[:, 8 * gg:8 * gg + 8],
                                          sfc[:, GS * gg:GS * (gg + 1)])
                        gsc = sp.tile([128, NG], F32, tag="gsc",
                                      name=f"gsc_{g}_{i}")
                        nc.vector.tensor_reduce(
                            gsc[:],
                            g8[:].rearrange("p (g i) -> p g i",
                                            i=8)[:, :, 0:2],
                            axis=AX.X, op=ALU.add)
                        gt8 = sp.tile([128, 8], F32, tag="gt8",
                                      name=f"gt8_{g}_{i}")
                        nc.vector.max(gt8[:], gsc[:])
                        pen = sp.tile([128, NG], F32, tag="pen",
                                      name=f"pen_{g}_{i}")
                        nc.vector.tensor_scalar(pen[:], gsc[:], gt8[:, 3:4],
                                                -BIG, op0=ALU.is_lt,
                                                op1=ALU.mult)
                        masked = rp.tile([128, E], F32, tag="masked",
                                         name=f"mk_{g}_{i}")
                        for gg in range(NG):
                            nc.gpsimd.tensor_scalar_add(
                                masked[:, GS * gg:GS * (gg + 1)],
                                sfc[:, GS * gg:GS * (gg + 1)],
                                pen[:, gg:gg + 1])
                        st["sfc"], st["masked"] = sfc, masked
                    elif piece == 2:
                        masked = st["masked"]
                        m8 = sp.tile([128, 8], F32, tag="m8",
                                     name=f"m8_{g}_{i}")
                        nc.vector.max(m8[:], masked[:])
                        if g not in gout:
                            gout[g] = (
                                sp.tile([128, NTT, 8], U32, tag="oidx",
                                        name=f"oidx_{g}"),
                                sp.tile([128, NTT, 8], F32, tag="owout",
                                        name=f"owout_{g}"))
                        i8 = gout[g][0][:, i, :]
                        nc.vector.max_index(i8, m8[:], masked[:])
                        i8f = sp.tile([128, 8], F32, tag="i8f",
                                      name=f"i8f_{g}_{i}")
                        nc.vector.tensor_copy(i8f[:], i8)
                        st["m8"], st["i8"], st["i8f"] = m8, i8, i8f
                    elif piece == 3:
                        i8f = st["i8f"]
                        junk = rp.tile([128, E], F32, tag="junk",
                                       name=f"junk_{g}_{i}")
                        biasg = sp.tile([128, 8], F32, tag="biasg",
                                        name=f"biasg_{g}_{i}")
                        for k in range(8):
                            nc.vector.scalar_tensor_tensor(
                                junk[:], iota_sb[:], i8f[:, k:k + 1],
                                bias_sb[:], op0=ALU.is_equal, op1=ALU.mult,
                                accum_out=biasg[:, k:k + 1])
                        st["biasg"] = biasg
                    else:
                        m8, i8, biasg = st["m8"], st["i8"], st["biasg"]
                        wraw = sp.tile([128, 8], F32, tag="wraw",
                                       name=f"wraw_{g}_{i}")
                        nc.vector.tensor_tensor(wraw[:], m8[:], biasg[:],
                                                op=ALU.subtract)
                        ssum = sp.tile([128, 1], F32, tag="ssum",
                                       name=f"ssum_{g}_{i}")
                        nc.vector.tensor_reduce(ssum[:], wraw[:], axis=AX.X,
                                                op=ALU.add)
                        inv = sp.tile([128, 1], F32, tag="inv",
                                      name=f"inv_{g}_{i}")
                        nc.vector.reciprocal(inv[:], ssum[:])
                        nc.vector.tensor_scalar(gout[g][1][:, i, :],
                                                wraw[:], inv[:],
                                                2.5, op0=ALU.mult,
                                                op1=ALU.mult)
                        if i == NTT - 1:  # one batched DMA per output
                            oidx, owout = gout.pop(g)
                            nc.sync.dma_start(
                                idx_d[TG * g:TG * (g + 1), :].rearrange(
                                    "(i p) k -> p i k", p=128),
                                oidx[:].bitcast(I32))
                            nc.sync.dma_start(
                                wout_d[TG * g:TG * (g + 1), :].rearrange(
                                    "(i p) k -> p i k", p=128),
                                owout[:])
                        rstate.pop((g, i))

                # ttile i pieces at steps 6+12i .. 10+12i
                ROUTE_AT = {6 + 12 * i + p: (i, p)
                            for i in range(NTT) for p in range(5)}
                for q in range(5):
                    w_load(q)
                for g in range(ngroups):
                    emit_x_dma(g)
                    main = [mainps.tile([128, 512], F32, tag=f"main{eh}",
                                        name=f"main_{g}_{eh}")
                            for eh in range(2)]
                    corr = [corrps.tile([128, 512], F32, tag=f"corr{eh}",
                                        name=f"corr_{g}_{eh}")
                            for eh in range(2)] if fp8c else None
                    for s in range(NCH + LAG + 1):
                        if s < NCH:
                            if g == 0 and s % 2 == 0 and 5 + s // 2 < NQ:
                                w_load(5 + s // 2)
                            emit_t_copies(g, s)
                        cg = s - LAG
                        if 0 <= cg < NCH:
                            emit_main(g, cg, main)
                        if fp8c and cg >= 2 and cg % 2 == 0:
                            emit_corr(cg // 2 - 1, corr)
                        if route_q and s in ROUTE_AT:
                            gq, ltT = route_q[0]
                            i, p = ROUTE_AT[s]
                            emit_routing_piece(gq, i, ltT, p)
                            if s == max(ROUTE_AT):
                                route_q.pop(0)
                    emit_combine(g, main, corr)
                # drain last group's routing
                gq, ltT = route_q.pop(0)
                for i in range(NTT):
                    for p in range(5):
                        emit_routing_piece(gq, i, ltT, p)

            if repeat == 1:
                emit_all()
            else:
                with tc.For_i(0, repeat, 1):
                    emit_all()

    nc.compile()
    return nc


_NC_CACHE = {}
_T_FULL = 16384
_N_CORES = 8
_GEMM = "fp8corr"
_WARM = [False]


def _run_on_device(hidden_states, weight, e_score_correction_bias):
    from concourse.bass_utils import run_bass_kernel_spmd

    x = np.ascontiguousarray(
        np.asarray(hidden_states, dtype=np.float32).reshape(_T_FULL, H))
    w = np.asarray(weight, dtype=np.float32)
    bias = np.asarray(e_score_correction_bias, dtype=np.float32)
    t_core = _T_FULL // _N_CORES

    if _GEMM not in _NC_CACHE:
        _NC_CACHE[_GEMM] = _build(t_core, gemm=_GEMM, n_devices=_N_CORES)
    nc = _NC_CACHE[_GEMM]

    base = {
        "wTr": np.ascontiguousarray(w.T),
        "bias_b": np.ascontiguousarray(
            np.broadcast_to(bias[None, :], (128, E))),
        "iota_b": np.ascontiguousarray(
            np.broadcast_to(np.arange(E, dtype=np.float32)[None, :],
                            (128, E))),
        "ident": np.eye(128, dtype=np.float32),
    }
    maps = []
    for c in range(_N_CORES):
        m = dict(base)
        m["x"] = np.ascontiguousarray(x[c * t_core:(c + 1) * t_core])
        maps.append(m)

    br = run_bass_kernel_spmd(nc, maps, list(range(_N_CORES)))
    idx = np.concatenate(
        [br.results[c]["idx_out"] for c in range(_N_CORES)],
        axis=0).astype(np.int32)
    wout = np.concatenate(
        [br.results[c]["w_out"] for c in range(_N_CORES)],
        axis=0).astype(np.float32)
    return idx, wout


def kernel(hidden_states, weight, e_score_correction_bias):
    """Crash-resilient driver: the very first dispatch after a fresh
    neuronxcc compile has been observed (rarely) to trip the accelerator
    (NRT_EXEC_UNIT_UNRECOVERABLE); a process restart with the warm
    compile cache always recovers. So the first call runs the dispatch
    in a child process; on failure it retries (cache now warm). Once one
    run has succeeded, later calls dispatch in-process (fast path)."""
    if _WARM[0]:
        return _run_on_device(hidden_states, weight,
                              e_score_correction_bias)
    import os
    import subprocess
    import tempfile
    me = os.path.abspath(__file__)
    try:
        tmpd = tempfile.mkdtemp(prefix="moegate_")
        inp = os.path.join(tmpd, "in.npz")
        outp = os.path.join(tmpd, "out.npz")
        np.savez(inp, hidden_states=np.asarray(hidden_states),
                 weight=np.asarray(weight),
                 e_score_correction_bias=np.asarray(
                     e_score_correction_bias))
        last = None
        for attempt in range(3):
            r = subprocess.run(
                [sys.executable, me, "--worker", inp, outp],
                cwd=os.path.dirname(me) or ".", timeout=1200,
                capture_output=True)
            if r.returncode == 0 and os.path.exists(outp):
                d = np.load(outp)
                _WARM[0] = True
                return (d["idx"].astype(np.int32),
                        d["wout"].astype(np.float32))
            last = r
        if last is not None:
            sys.stderr.write(last.stderr.decode(errors="replace")[-2000:])
    except Exception:
        pass
    # fallback: dispatch in-process (with one retry)
    try:
        out = _run_on_device(hidden_states, weight,
                             e_score_correction_bias)
    except Exception:
        out = _run_on_device(hidden_states, weight,
                             e_score_correction_bias)
    _WARM[0] = True
    return out


if __name__ == "__main__" and len(sys.argv) == 4 and sys.argv[1] == "--worker":
    _d = np.load(sys.argv[2])
    _idx, _wout = _run_on_device(_d["hidden_states"], _d["weight"],
                                 _d["e_score_correction_bias"])
    np.savez(sys.argv[3], idx=_idx, wout=_wout)
